# revision 1
# baseline (speedup 1.0000x reference)
"""Trainium2 Bass kernel for DilatedReparamConv (5-branch depthwise conv + BN + SiLU + identity BN).

out = BN_id(x) + sum_i silu(BN_i(dwconv_i(x)))   for branches
      (5,d1), (7,d2), (3,d3), (3,d4), (3,d5), all SAME padding.

Strategy (8 NeuronCores, SPMD):
  - Shard the 256 channels across 8 cores (32 ch/core, all 32 images).
  - Depthwise conv on TensorE: for each channel-pair, a block-diagonal banded
    Toeplitz matrix T[(ci,h_in),(ci,h_out)] contracts the whole kh tap-stack in
    one matmul; kw taps become free-dim shifts into a W-padded x tile and
    accumulate in PSUM (one matmul per kw tap, start/stop flags).
  - BN+SiLU fused into the ScalarE PSUM->SBUF eviction (per-partition
    scale/bias APs). Branch accumulation on VectorE. Identity branch on
    VectorE tensor_scalar.
  - Host precomputes the banded matrices and padded layouts so every DMA is
    one contiguous block per channel-pair.

MM_DTYPE:
  - "bf16":  bf16 matmul inputs, 1 PE cycle/row.
  - "f32r":  fp32 storage, PE truncates to FP22. Most accurate, ~10% slower.
  - "fp8dr": e4m3 + DoubleRow (0.5 cycle/row). The two per-partition slots
    carry (x8, residual) x (T8, T8/2) error-feedback pairs, so x is corrected
    to ~0.1%: out = T8*x8 + (T8/2)*(2*(x-x8)) = T8*x. Free dim is w-major
    (w,b) so the [K,2,N] rhs AP is contiguous.
"""

import sys

sys.path.insert(0, "/opt/trn_rl_repo")

import numpy as np
import ml_dtypes

import concourse.bass as bass
import concourse.mybir as mybir
from concourse import bacc, tile
from concourse.bass_utils import run_bass_kernel_spmd

# ---------------------------------------------------------------- problem dims
B, C, H, W = 32, 256, 64, 64
EPS = 1e-5
BRANCH_CFG = [(5, 1), (7, 2), (3, 3), (3, 4), (3, 5)]  # (kernel, dilation)
N_CORES = 8
C_CORE = C // N_CORES          # 32 channels per core
PAIRS = C_CORE // 2            # 16 channel-pairs per core
PAD = 6                        # max dilation*(ks-1)//2 across branches
WP = W + 2 * PAD               # padded width = 76
NTAPS = sum(ks for ks, _ in BRANCH_CFG)   # 21 kw taps total

MM_DTYPE = "bf16"              # "bf16" | "f32r" | "fp8dr"

F8 = ml_dtypes.float8_e4m3
BF16 = ml_dtypes.bfloat16

# fp8dr packed per-partition byte layout (all uint8 columns)
X8_OFF = 0                     # x8 plane + residual plane, [2, WP*B] fp8
XN_OFF = 2 * WP * B            # natural x, [W*B] bf16 -> 2*W*B bytes
W8_OFF = XN_OFF + 2 * W * B    # Toeplitz, [NTAPS, 2, 128] fp8
FP8_COLS = W8_OFF + NTAPS * 2 * 128

_CACHE: dict = {}


def _taps():
    """(j, branch_idx, dx) for the 21 kw taps, branch-major."""
    out = []
    j = 0
    for br, (ks, dil) in enumerate(BRANCH_CFG):
        pad = dil * (ks - 1) // 2
        for kw in range(ks):
            out.append((j, br, dil * kw - pad))
            j += 1
    return out


def _br_tap_ranges():
    out, j0 = [], 0
    for ks, _ in BRANCH_CFG:
        out.append((j0, j0 + ks))
        j0 += ks
    return out


# =====================================================================
# bf16 / f32r build (free dim = (b, w), img-chunked matmuls)
# =====================================================================
def build_nc_bf16():
    nc = bacc.Bacc("TRN2", target_bir_lowering=False, debug=False, num_devices=N_CORES)
    dt_in = mybir.dt.bfloat16 if MM_DTYPE == "bf16" else mybir.dt.float32r
    f32 = mybir.dt.float32
    # w-major free layout (w, b): each matmul's 512 moving columns are one
    # contiguous run (strided 2-D rhs APs cost ~10ns/matmul in segment overhead)
    W_CHUNK, N_CHUNKS = 16, 4

    XCOLS = B * WP
    XWCOLS = XCOLS + NTAPS * 128
    xw = nc.dram_tensor("xw", [PAIRS, 128, XWCOLS], dt_in, kind="ExternalInput").ap()
    scbi = nc.dram_tensor("scbi", [128, 2 * PAIRS * 6], f32, kind="ExternalInput").ap()
    yt = nc.dram_tensor("yt", [PAIRS, 128, B * W], f32, kind="ExternalOutput").ap()

    taps = _taps()
    ranges = _br_tap_ranges()

    with tile.TileContext(nc) as tc:
        with (
            tc.tile_pool(name="consts", bufs=1) as consts,
            tc.tile_pool(name="xwp", bufs=3) as xwp,
            tc.tile_pool(name="accp", bufs=2) as accp,
            tc.tile_pool(name="tp", bufs=3) as tp,
            tc.tile_pool(name="psum", bufs=6, space="PSUM") as psum,
        ):
            scbi_t = consts.tile([128, 2 * PAIRS * 6], f32)
            nc.sync.dma_start(out=scbi_t[:], in_=scbi)
            sc_t = scbi_t[:, : PAIRS * 6]
            bi_t = scbi_t[:, PAIRS * 6 :]

            for p in range(PAIRS):
                xw_t = xwp.tile([128, XWCOLS], dt_in)
                nc.sync.dma_start(out=xw_t[:], in_=xw[p])
                xt_t = xw_t[:, :XCOLS]
                wt_t = xw_t[:, XCOLS:]

                acc = accp.tile([128, B * W], f32)
                xr = xt_t.rearrange("p (w b) -> p w b", b=B)

                acc3 = acc.rearrange("p (w b) -> p w b", b=B)
                xr_id = xr[:, PAD : PAD + W, :]
                if MM_DTYPE == "f32r":
                    xr_id = xr_id.bitcast(mybir.dt.float32)
                nc.vector.tensor_scalar(
                    acc3[:, :, :],
                    xr_id,
                    sc_t[:, p * 6 + 5 : p * 6 + 6],
                    bi_t[:, p * 6 + 5 : p * 6 + 6],
                    mybir.AluOpType.mult,
                    mybir.AluOpType.add,
                )

                for br in range(5):
                    jlo, jhi = ranges[br]
                    t_full = tp.tile([128, B * W], f32)
                    for cch in range(N_CHUNKS):
                        ps = psum.tile([128, W_CHUNK * B], f32)
                        for j, _br, dx in taps[jlo:jhi]:
                            base = (cch * W_CHUNK + PAD + dx) * B
                            rhs = xt_t[:, base : base + W_CHUNK * B]
                            lhsT = wt_t[:, j * 128 : (j + 1) * 128]
                            nc.tensor.matmul(
                                ps[:], lhsT, rhs,
                                start=(j == jlo), stop=(j == jhi - 1),
                            )
                        nc.scalar.activation(
                            t_full[:, cch * W_CHUNK * B : (cch + 1) * W_CHUNK * B],
                            ps[:],
                            mybir.ActivationFunctionType.Silu,
                            bias=bi_t[:, p * 6 + br : p * 6 + br + 1],
                            scale=sc_t[:, p * 6 + br : p * 6 + br + 1],
                        )
                    nc.vector.tensor_tensor(
                        acc[:], acc[:], t_full[:], op=mybir.AluOpType.add
                    )

                nc.sync.dma_start(out=yt[p], in_=acc[:])

    nc.compile()
    return nc


# =====================================================================
# fp8 DoubleRow build (free dim = (w, b), w-chunked matmuls)
# =====================================================================
def build_nc_fp8():
    nc = bacc.Bacc("TRN2", target_bir_lowering=False, debug=False, num_devices=N_CORES)
    f32 = mybir.dt.float32
    u8 = mybir.dt.uint8
    f8 = mybir.dt.float8e4
    W_CHUNK, N_CHUNKS = 16, 4   # w-columns per psum chunk; N = 16*32 = 512

    xw = nc.dram_tensor("xw", [PAIRS, 128, FP8_COLS], u8, kind="ExternalInput").ap()
    scbi = nc.dram_tensor("scbi", [128, 2 * PAIRS * 6], f32, kind="ExternalInput").ap()
    yt = nc.dram_tensor("yt", [PAIRS, 128, B * W], f32, kind="ExternalOutput").ap()

    taps = _taps()
    ranges = _br_tap_ranges()

    with tile.TileContext(nc) as tc:
        with (
            tc.tile_pool(name="consts", bufs=1) as consts,
            tc.tile_pool(name="xwp", bufs=3) as xwp,
            tc.tile_pool(name="accp", bufs=2) as accp,
            tc.tile_pool(name="tp", bufs=3) as tp,
            tc.tile_pool(name="psum", bufs=6, space="PSUM") as psum,
        ):
            scbi_t = consts.tile([128, 2 * PAIRS * 6], f32)
            nc.sync.dma_start(out=scbi_t[:], in_=scbi)
            sc_t = scbi_t[:, : PAIRS * 6]
            bi_t = scbi_t[:, PAIRS * 6 :]

            for p in range(PAIRS):
                xw_t = xwp.tile([128, FP8_COLS], u8)
                nc.sync.dma_start(out=xw_t[:], in_=xw[p])
                # [128, 2, WP*B] fp8: slot 0 = x8, slot 1 = 2*(x - x8)
                x8r = xw_t[:, X8_OFF:XN_OFF].bitcast(f8).rearrange(
                    "p (r q) -> p r q", r=2
                )
                # [128, W*B] bf16 natural x for the identity branch
                xnat = xw_t[:, XN_OFF:W8_OFF].bitcast(mybir.dt.bfloat16)
                # [128, NTAPS, 2, 128] fp8: slot 0 = T8, slot 1 = T8/2
                wt8 = xw_t[:, W8_OFF:].bitcast(f8).rearrange(
                    "p (j r m) -> p j r m", j=NTAPS, r=2
                )

                acc = accp.tile([128, B * W], f32)
                nc.vector.tensor_scalar(
                    acc[:],
                    xnat,
                    sc_t[:, p * 6 + 5 : p * 6 + 6],
                    bi_t[:, p * 6 + 5 : p * 6 + 6],
                    mybir.AluOpType.mult,
                    mybir.AluOpType.add,
                )

                for br in range(5):
                    jlo, jhi = ranges[br]
                    t_full = tp.tile([128, B * W], f32)
                    for cch in range(N_CHUNKS):
                        ps = psum.tile([128, W_CHUNK * B], f32)
                        for j, _br, dx in taps[jlo:jhi]:
                            base = (cch * W_CHUNK + PAD + dx) * B
                            rhs = x8r[:, :, base : base + W_CHUNK * B]
                            nc.tensor.matmul(
                                ps[:], wt8[:, j], rhs,
                                start=(j == jlo), stop=(j == jhi - 1),
                                perf_mode=mybir.MatmulPerfMode.DoubleRowSwInterleave,
                            )
                        nc.scalar.activation(
                            t_full[:, cch * W_CHUNK * B : (cch + 1) * W_CHUNK * B],
                            ps[:],
                            mybir.ActivationFunctionType.Silu,
                            bias=bi_t[:, p * 6 + br : p * 6 + br + 1],
                            scale=sc_t[:, p * 6 + br : p * 6 + br + 1],
                        )
                    nc.vector.tensor_tensor(
                        acc[:], acc[:], t_full[:], op=mybir.AluOpType.add
                    )

                nc.sync.dma_start(out=yt[p], in_=acc[:])

    nc.compile()
    return nc


# ------------------------------------------------------------------ host prep
def _bn_scale_bias(gamma, beta, mean, var):
    s = gamma / np.sqrt(var + EPS)
    return s, beta - mean * s


def _toeplitz(weights):
    """T[c, j, hi, ho] banded matrices for all 21 taps."""
    T = np.zeros((C, NTAPS, H, H), np.float32)
    j = 0
    for br, (ks, dil) in enumerate(BRANCH_CFG):
        pad = dil * (ks - 1) // 2
        wbr = weights[br][:, 0]  # [C, ks, ks]
        for kw in range(ks):
            for kh in range(ks):
                off = dil * kh - pad
                ho = np.arange(max(0, -off), min(H, H - off))
                T[:, j, ho + off, ho] = wbr[:, kh, kw][:, None]
            j += 1
    return T


def _bn_tables(id_bn, bn_gamma, bn_beta, bn_mean, bn_var, gain=None):
    """S[6, C], Bv[6, C]; branch scales divided by per-(branch,channel) gain."""
    S = np.zeros((6, C), np.float32)
    Bv = np.zeros((6, C), np.float32)
    for i in range(5):
        S[i], Bv[i] = _bn_scale_bias(bn_gamma[i], bn_beta[i], bn_mean[i], bn_var[i])
        if gain is not None:
            S[i] = S[i] / gain[i]
    S[5], Bv[5] = _bn_scale_bias(id_bn[0], id_bn[1], id_bn[2], id_bn[3])
    return S, Bv


def _scbi_cores(S, Bv):
    out = []
    for k in range(N_CORES):
        sck = np.empty((128, PAIRS * 6), np.float32)
        bik = np.empty((128, PAIRS * 6), np.float32)
        for p in range(PAIRS):
            for i in range(6):
                for ci in range(2):
                    c = k * C_CORE + 2 * p + ci
                    sck[ci * H : (ci + 1) * H, p * 6 + i] = S[i, c]
                    bik[ci * H : (ci + 1) * H, p * 6 + i] = Bv[i, c]
        out.append(np.ascontiguousarray(np.concatenate([sck, bik], axis=1)))
    return out


def _host_prep_bf16(x, weights, S, Bv):
    mmdt = BF16 if MM_DTYPE == "bf16" else np.float32
    # w-major free layout: [C, H, WP, B]
    xt_full = np.zeros((C, H, WP, B), np.float32)
    xt_full[:, :, PAD : PAD + W, :] = np.transpose(x, (1, 2, 3, 0))
    T = _toeplitz(weights)
    Tr = T.reshape(N_CORES, PAIRS, 2, NTAPS, H, H)
    scbi = _scbi_cores(S, Bv)

    in_maps = []
    for k in range(N_CORES):
        xs = xt_full[k * C_CORE : (k + 1) * C_CORE].reshape(PAIRS, 2 * H, B * WP)
        wmk = np.zeros((PAIRS, 128, NTAPS, 128), np.float32)
        for ci in range(2):
            wmk[:, ci * H : (ci + 1) * H, :, ci * H : (ci + 1) * H] = np.transpose(
                Tr[k, :, ci], (0, 2, 1, 3)
            )
        xwk = np.concatenate(
            [xs.astype(mmdt), wmk.reshape(PAIRS, 128, NTAPS * 128).astype(mmdt)],
            axis=2,
        )
        in_maps.append({"xw": np.ascontiguousarray(xwk), "scbi": scbi[k]})
    return in_maps


def _host_prep_fp8(x, weights, id_bn, bn_gamma, bn_beta, bn_mean, bn_var):
    # per-(branch, channel) gain so T8 uses the e4m3 range well
    gain = np.empty((5, C), np.float32)
    for br in range(5):
        wmax = np.abs(weights[br][:, 0]).max(axis=(1, 2))
        gain[br] = 8.0 / np.maximum(wmax, 1e-8)
    S, Bv = _bn_tables(id_bn, bn_gamma, bn_beta, bn_mean, bn_var, gain=gain)
    scbi = _scbi_cores(S, Bv)

    # x in [C, H, WP, B] (w-major, batch innermost)
    xp = np.zeros((C, H, WP, B), np.float32)
    xp[:, :, PAD : PAD + W, :] = np.transpose(x, (1, 2, 3, 0))
    x8 = xp.astype(F8)
    r8 = (2.0 * (xp - x8.astype(np.float32))).astype(F8)
    # natural x (bf16) in [C, H, W, B]
    xn = np.ascontiguousarray(
        np.transpose(x, (1, 2, 3, 0)).astype(BF16)
    )

    T = _toeplitz(weights)  # [C, NTAPS, hi, ho]
    jr = _br_tap_ranges()
    for br in range(5):
        T[:, jr[br][0] : jr[br][1]] *= gain[br][:, None, None, None]
    T8 = T.astype(F8)
    T8q = (T8.astype(np.float32) / 2.0).astype(F8)

    in_maps = []
    for k in range(N_CORES):
        sl = slice(k * C_CORE, (k + 1) * C_CORE)
        xwk = np.zeros((PAIRS, 128, FP8_COLS), np.uint8)
        # x8 | r8 planes
        planes = np.stack([x8[sl], r8[sl]], axis=2)  # [32, H, 2, WP, B]
        planes = planes.reshape(PAIRS, 2 * H, 2 * WP * B)
        xwk[:, :, X8_OFF:XN_OFF] = planes.view(np.uint8)
        # natural bf16
        xwk[:, :, XN_OFF:W8_OFF] = (
            xn[sl].reshape(PAIRS, 2 * H, W * B).view(np.uint8)
        )
        # Toeplitz block-diag [K, NTAPS, 2, M]
        wmk = np.zeros((PAIRS, 128, NTAPS, 2, 128), F8)
        T8k = T8[sl].reshape(PAIRS, 2, NTAPS, H, H)
        T8qk = T8q[sl].reshape(PAIRS, 2, NTAPS, H, H)
        for ci in range(2):
            blk = slice(ci * H, (ci + 1) * H)
            wmk[:, blk, :, 0, blk] = np.transpose(T8k[:, ci], (0, 2, 1, 3))
            wmk[:, blk, :, 1, blk] = np.transpose(T8qk[:, ci], (0, 2, 1, 3))
        # DoubleRowSwInterleave weight layout per partition row:
        # [A127, B127, A126, B126, ..., A0, B0] (A/B pairs, columns reversed)
        wmk = wmk[:, :, :, :, ::-1]                  # reverse M columns
        wmk = np.swapaxes(wmk, 3, 4)                 # [.., M, 2] -> interleave A/B
        xwk[:, :, W8_OFF:] = wmk.reshape(PAIRS, 128, NTAPS * 2 * 128).view(np.uint8)
        in_maps.append({"xw": np.ascontiguousarray(xwk), "scbi": scbi[k]})
    return in_maps


def _host_prep(x, id_bn, w5, w7, w3a, w3b, w3c, bn_gamma, bn_beta, bn_mean, bn_var):
    x = np.asarray(x, np.float32)
    weights = [np.asarray(w, np.float32) for w in (w5, w7, w3a, w3b, w3c)]
    args = [np.asarray(a, np.float32) for a in (id_bn, bn_gamma, bn_beta, bn_mean, bn_var)]
    if MM_DTYPE == "fp8dr":
        return _host_prep_fp8(x, weights, *args)
    S, Bv = _bn_tables(*args)
    return _host_prep_bf16(x, weights, S, Bv)


def _assemble(results):
    y = np.empty((B, C, H, W), np.float32)
    for k in range(N_CORES):
        ytk = np.asarray(results[k]["yt"], np.float32)  # [PAIRS, 128, B*W], (w,b) cols
        ytk = ytk.reshape(PAIRS, 2, H, W, B).transpose(4, 0, 1, 2, 3)
        y[:, k * C_CORE : (k + 1) * C_CORE] = ytk.reshape(B, C_CORE, H, W)
    return y


def kernel_run(inputs, trace=False, tmpdir=None):
    if "nc" not in _CACHE:
        _CACHE["nc"] = build_nc_fp8() if MM_DTYPE == "fp8dr" else build_nc_bf16()
    nc = _CACHE["nc"]
    in_maps = _host_prep(**inputs)
    res = run_bass_kernel_spmd(
        nc, in_maps, list(range(N_CORES)), trace=trace, tmpdir=tmpdir
    )
    return _assemble(res.results), res


def kernel(**inputs):
    out, _ = kernel_run(inputs, trace=False)
    return out

